# revision 1
# baseline (speedup 1.0000x reference)
"""Local (windowed) attention kernel for TRN2, 8 NeuronCores, SPMD.

Reference computation (B=4, N=8192, DIM=1024, H=16, DH=64, W=128):
    q = x @ wq ; k,v = split(x @ wkv)
    per (batch, head, window of 128): attend to [prev window, cur window]
    with causal mask (j > i + W masked), softmax, out = attn @ v
    out = out @ wo + bo

Sharding: sequence dim split into 8 contiguous chunks of 1024 tokens, one
per core.  Each core receives its x slice with a 128-token halo in front
(zeros for core 0 — matches the reference's zero-pad of the first window)
and computes q/k/v projections, attention, and the output projection for
its own tokens only.  Weights are replicated.  No collectives.

On-chip layout is feature-major (x fed pre-transposed as [dim, token]),
so every matmul uses natural HBM weight layouts and no on-chip
transposes are needed except the attention-probs transpose (done on PE).
All matmuls run in bf16 (1 cycle/row on the PE) with fp32 PSUM
accumulation; softmax runs in fp32.  exp() is computed without
max-subtraction: scores are O(1) here (|s| < ~4), so overflow is
impossible and this matches jax.nn.softmax to fp32 roundoff.
"""

import numpy as np
import ml_dtypes

import concourse.bass as bass
import concourse.bacc as bacc
import concourse.mybir as mybir
import concourse.tile as tile
from concourse.bass_utils import run_bass_kernel_spmd

B, N, DIM = 4, 8192, 1024
H, DH, W = 16, 64, 128
NCORES = 8
TOW = N // NCORES          # own tokens per core per batch   = 1024
TH = TOW + W               # with front halo                 = 1152
NW = TOW // W              # own windows per core-batch      = 8
KT = DIM // 128            # contraction tiles               = 8
MT = DIM // 128            # inner/output tiles              = 8
SCALE = DH ** -0.5

BF16 = mybir.dt.bfloat16
F32 = mybir.dt.float32
AX = mybir.AxisListType
AF = mybir.ActivationFunctionType

TRACE = False              # set by test.py to collect an NTFF profile
TRACE_KW = {}
LAST_RESULT = None         # BassKernelResults stash when TRACE
REPEAT = 1                 # whole-computation repeats inside the NEFF (bench)


def _build_bass():
    nc = bacc.Bacc(None, target_bir_lowering=False)
    xT = nc.declare_dram_parameter("xT", [B, DIM, TH], BF16, isOutput=False)
    wq = nc.declare_dram_parameter("wq", [DIM, DIM], BF16, isOutput=False)
    wkv = nc.declare_dram_parameter("wkv", [DIM, 2 * DIM], BF16, isOutput=False)
    wo = nc.declare_dram_parameter("wo", [DIM, DIM], BF16, isOutput=False)
    bo_pm = nc.declare_dram_parameter("bo_pm", [128, MT], F32, isOutput=False)
    maskT = nc.declare_dram_parameter("maskT", [128, 128], BF16, isOutput=False)
    ident = nc.declare_dram_parameter("ident", [128, 128], BF16, isOutput=False)
    outT = nc.declare_dram_parameter("outT", [B, DIM, TOW], F32, isOutput=True)

    with tile.TileContext(nc) as tc:
        with (
            tc.tile_pool(name="wpool", bufs=1) as wpool,
            tc.tile_pool(name="xpool", bufs=2) as xpool,
            tc.tile_pool(name="actpool", bufs=1) as actpool,
            tc.tile_pool(name="spool", bufs=2) as spool,
            tc.tile_pool(name="opool", bufs=3) as opool,
            tc.tile_pool(name="pscores", bufs=1, space="PSUM") as pscores,
            tc.tile_pool(name="pattnT", bufs=1, space="PSUM") as pattnT,
            tc.tile_pool(name="ppv", bufs=1, space="PSUM") as ppv,
            tc.tile_pool(name="pproj", bufs=2, space="PSUM") as pproj,
        ):
            # ---- replicated constants (loaded once) ----
            wq_sb = wpool.tile([128, KT, DIM], BF16)
            wkv_sb = wpool.tile([128, KT, 2 * DIM], BF16)
            wo_sb = wpool.tile([128, KT, DIM], BF16)
            bo_sb = wpool.tile([128, MT], F32)
            mask_sb = wpool.tile([128, 128], BF16)
            id_sb = wpool.tile([128, 128], BF16)
            for k in range(KT):
                nc.sync.dma_start(out=wq_sb[:, k, :], in_=wq[k * 128:(k + 1) * 128, :])
                nc.sync.dma_start(out=wkv_sb[:, k, :], in_=wkv[k * 128:(k + 1) * 128, :])
                nc.sync.dma_start(out=wo_sb[:, k, :], in_=wo[k * 128:(k + 1) * 128, :])
            nc.sync.dma_start(out=bo_sb, in_=bo_pm[:])
            nc.sync.dma_start(out=mask_sb, in_=maskT[:])
            nc.sync.dma_start(out=id_sb, in_=ident[:])

            for b in [bb % B for bb in range(B * REPEAT)]:
                # ---- load xT slice (feature-major, with halo) ----
                x_sb = xpool.tile([128, KT, TH], BF16, tag="x")
                nc.gpsimd.dma_start(
                    out=x_sb[:],
                    in_=xT[b].rearrange("(k p) t -> p k t", p=128),
                )

                qT = actpool.tile([128, MT, TOW], BF16, tag="qT")
                kTt = actpool.tile([128, MT, TH], BF16, tag="kT")
                v_sb = actpool.tile([128, NW + 1, DIM], BF16, tag="v")
                aoT = actpool.tile([128, MT, TOW], BF16, tag="aoT")

                # ---- q projection, feature-major: qT[m] = wq[:,m].T @ x ----
                for m in range(MT):
                    for c in range(2):
                        ps = pproj.tile([128, 512], F32, tag="proj")
                        for k in range(KT):
                            nc.tensor.matmul(
                                ps,
                                lhsT=wq_sb[:, k, m * 128:(m + 1) * 128],
                                rhs=x_sb[:, k, W + c * 512:W + (c + 1) * 512],
                                start=(k == 0),
                                stop=(k == KT - 1),
                            )
                        nc.vector.tensor_copy(
                            out=qT[:, m, c * 512:(c + 1) * 512], in_=ps
                        )

                # ---- k projection, feature-major (incl. halo) ----
                for m in range(MT):
                    for c in range(3):
                        ps = pproj.tile([128, 384], F32, tag="proj")
                        for k in range(KT):
                            nc.tensor.matmul(
                                ps,
                                lhsT=wkv_sb[:, k, m * 128:(m + 1) * 128],
                                rhs=x_sb[:, k, c * 384:(c + 1) * 384],
                                start=(k == 0),
                                stop=(k == KT - 1),
                            )
                        nc.vector.tensor_copy(
                            out=kTt[:, m, c * 384:(c + 1) * 384], in_=ps
                        )

                # ---- v projection, token-major (incl. halo) ----
                for wi in range(NW + 1):
                    for c in range(2):
                        ps = pproj.tile([128, 512], F32, tag="proj")
                        for k in range(KT):
                            nc.tensor.matmul(
                                ps,
                                lhsT=x_sb[:, k, wi * 128:(wi + 1) * 128],
                                rhs=wkv_sb[:, k, DIM + c * 512:DIM + (c + 1) * 512],
                                start=(k == 0),
                                stop=(k == KT - 1),
                            )
                        nc.vector.tensor_copy(
                            out=v_sb[:, wi, c * 512:(c + 1) * 512], in_=ps
                        )

                # ---- attention: 8 windows x 4 groups of 4 heads ----
                for w in range(NW):
                    for g in range(4):
                        sc = pscores.tile([128, 4, 512], F32, tag="scores")
                        for hh in range(4):
                            h = 4 * g + hh
                            m, r = h // 2, (h % 2) * 64
                            nc.tensor.matmul(
                                sc[:, hh, 0:2 * W],
                                lhsT=qT[r:r + 64, m, w * W:(w + 1) * W],
                                rhs=kTt[r:r + 64, m, w * W:w * W + 2 * W],
                                start=True,
                                stop=True,
                            )
                            # additive causal mask on the current-window half:
                            # sc[:, hh, W+jc] += maskT[jc, i]  (-1e30 where jc > i)
                            nc.tensor.matmul(
                                sc[:, hh, W:2 * W],
                                lhsT=mask_sb,
                                rhs=id_sb,
                                start=False,
                                stop=False,
                                skip_group_check=True,
                            )
                        # softmax (no max-subtraction; scores are O(1));
                        # exp + per-head row-sum fused on ACT via accum_out
                        exps = spool.tile([128, 4, 2 * W], F32, tag="expS")
                        sums = spool.tile([128, 4], F32, tag="sums")
                        for hh in range(4):
                            nc.scalar.activation(
                                out=exps[:, hh, :],
                                in_=sc[:, hh, 0:2 * W],
                                func=AF.Exp,
                                bias=0.0,
                                scale=SCALE,
                                accum_out=sums[:, hh:hh + 1],
                            )
                        recip = spool.tile([128, 4], F32, tag="recip")
                        nc.vector.reciprocal(out=recip, in_=sums)
                        attn = spool.tile([128, 4, 2 * W], BF16, tag="attnb")
                        for hh in range(4):
                            nc.scalar.activation(
                                out=attn[:, hh, :],
                                in_=exps[:, hh, :],
                                func=AF.Copy,
                                bias=0.0,
                                scale=recip[:, hh:hh + 1],
                            )
                        # transpose probs on PE: [i, j] -> [j, i]
                        pT = pattnT.tile([128, 8, 128], BF16, tag="attnT")
                        for hh in range(4):
                            for hf in range(2):
                                nc.tensor.transpose(
                                    out=pT[:, hh * 2 + hf, :],
                                    in_=attn[:, hh, hf * W:(hf + 1) * W],
                                    identity=id_sb,
                                )
                        aT = spool.tile([128, 8, 128], BF16, tag="attnT_sb")
                        nc.vector.tensor_copy(out=aT, in_=pT)
                        # pv: outT[dh, i] = v[j, dh].T @ attnT[j, i]
                        pv = ppv.tile([128, 2, 128], F32, tag="pv")
                        for hh in range(4):
                            h = 4 * g + hh
                            pr, pc = (hh % 2) * 64, hh // 2
                            for hf in range(2):
                                nc.tensor.matmul(
                                    pv[pr:pr + 64, pc, :],
                                    lhsT=v_sb[:, w + hf, h * 64:(h + 1) * 64],
                                    rhs=aT[:, hh * 2 + hf, :],
                                    start=(hf == 0),
                                    stop=(hf == 1),
                                )
                        for pc in range(2):
                            nc.vector.tensor_copy(
                                out=aoT[:, 2 * g + pc, w * W:(w + 1) * W],
                                in_=pv[:, pc, :],
                            )

                # ---- output projection + bias, feature-major ----
                for m in range(MT):
                    for c in range(2):
                        ps = pproj.tile([128, 512], F32, tag="proj")
                        for k in range(KT):
                            nc.tensor.matmul(
                                ps,
                                lhsT=wo_sb[:, k, m * 128:(m + 1) * 128],
                                rhs=aoT[:, k, c * 512:(c + 1) * 512],
                                start=(k == 0),
                                stop=(k == KT - 1),
                            )
                        osb = opool.tile([128, 512], F32, tag="outsb")
                        nc.vector.tensor_scalar_add(
                            out=osb, in0=ps, scalar1=bo_sb[:, m:m + 1]
                        )
                        nc.sync.dma_start(
                            out=outT[b, m * 128:(m + 1) * 128, c * 512:(c + 1) * 512],
                            in_=osb,
                        )
    nc.compile()
    return nc


_NC_CACHE = None


def _get_nc():
    global _NC_CACHE
    if _NC_CACHE is None:
        _NC_CACHE = _build_bass()
    return _NC_CACHE


def kernel(x, wq, wkv, wo, bo):
    global LAST_RESULT
    bfd = ml_dtypes.bfloat16
    x = np.asarray(x, np.float32)
    wq_b = np.asarray(wq, np.float32).astype(bfd)
    wkv_b = np.asarray(wkv, np.float32).astype(bfd)
    wo_b = np.asarray(wo, np.float32).astype(bfd)
    bo_pm = np.ascontiguousarray(
        np.asarray(bo, np.float32).reshape(MT, 128).T
    )
    # maskT[jc, i] = -1e30 where current-window col jc > row i (causal)
    maskT = np.where(
        np.arange(W)[:, None] > np.arange(W)[None, :], -1e30, 0.0
    ).astype(bfd)
    ident = np.eye(128, dtype=bfd)

    xb = x.astype(bfd)
    in_maps = []
    for c in range(NCORES):
        lo, hi = c * TOW - W, (c + 1) * TOW
        if c == 0:
            sl = np.concatenate(
                [np.zeros((B, W, DIM), bfd), xb[:, :hi]], axis=1
            )
        else:
            sl = xb[:, lo:hi]
        xT_c = np.ascontiguousarray(sl.transpose(0, 2, 1))  # [B, DIM, TH]
        in_maps.append(
            dict(xT=xT_c, wq=wq_b, wkv=wkv_b, wo=wo_b, bo_pm=bo_pm,
                 maskT=maskT, ident=ident)
        )

    nc = _get_nc()
    res = run_bass_kernel_spmd(
        nc, in_maps, list(range(NCORES)), trace=TRACE, **TRACE_KW
    )
    if TRACE:
        LAST_RESULT = res
    out = np.empty((B, N, DIM), np.float32)
    for c in range(NCORES):
        out[:, c * TOW:(c + 1) * TOW, :] = res.results[c]["outT"].transpose(0, 2, 1)
    return out



# revision 22
# speedup vs baseline: 1.3532x; 1.3532x over previous
"""Local (windowed) attention kernel for TRN2, 8 NeuronCores, SPMD.

Reference computation (B=4, N=8192, DIM=1024, H=16, DH=64, W=128):
    q = x @ wq ; k,v = split(x @ wkv)
    per (batch, head, window of 128): attend to [prev window, cur window]
    with causal mask (j > i + W masked), softmax, out = attn @ v
    out = out @ wo + bo

Sharding: sequence dim split into 8 contiguous chunks of 1024 tokens, one
per core.  Each core receives its x slice with a 128-token halo in front
(zeros for core 0 — matches the reference's zero-pad of the first window)
and computes q/k/v projections, attention, and the output projection for
its own tokens only.  Weights are replicated.  No collectives.

Attention is computed in the transposed orientation: scores come out of
the PE as S^T[j, i] (lhsT = k^T), so the probabilities are already laid
out with the contraction dim (j) on partitions for the PV matmul and no
per-window probability transposes are needed.  The softmax denominator
rides the PV matmul as an extra all-ones column appended to V; the
normalization (1/sum) is applied as a per-partition scalar while copying
the PV result (token-major) out of PSUM.  The attention output is
transposed back to feature-major (64 tiles/batch) for the output
projection.  exp() needs no max-subtraction: scores are O(1) here, so
overflow is impossible and this matches jax.nn.softmax to fp32 roundoff.
All matmuls run in bf16 (1 cycle/row on the PE) with fp32 PSUM
accumulation.
"""

import numpy as np
import ml_dtypes

import concourse.bass as bass
import concourse.bacc as bacc
import concourse.mybir as mybir
import concourse.tile as tile
from concourse.bass_utils import run_bass_kernel_spmd

B, N, DIM = 4, 8192, 1024
H, DH, W = 16, 64, 128
NCORES = 8
TOW = N // NCORES          # own tokens per core per batch   = 1024
TH = TOW + W               # with front halo                 = 1152
NW = TOW // W              # own windows per core-batch      = 8
KT = DIM // 128            # contraction tiles               = 8
MT = DIM // 128            # inner/output tiles              = 8
SCALE = DH ** -0.5

BF16 = mybir.dt.bfloat16
F32 = mybir.dt.float32
AX = mybir.AxisListType
AF = mybir.ActivationFunctionType

# debug feature gates
ATT_SC = True              # emit attention scores + exp
ATT_PV = True              # emit pv + recip + normalize
ATT_TP = True              # emit ao transposes + copy
OUTPROJ = True             # emit real output projection (else qT fallback)

TRACE = False              # set by test.py to collect an NTFF profile
TRACE_KW = {}
LAST_RESULT = None         # BassKernelResults stash when TRACE
REPEAT = 1                 # whole-computation repeats inside the NEFF (bench)


def _build_bass():
    nc = bacc.Bacc(None, target_bir_lowering=False)
    xT = nc.declare_dram_parameter("xT", [B, DIM, TH], BF16, isOutput=False)
    wq = nc.declare_dram_parameter("wq", [DIM, DIM], BF16, isOutput=False)
    wkv = nc.declare_dram_parameter("wkv", [DIM, 2 * DIM], BF16, isOutput=False)
    wo = nc.declare_dram_parameter("wo", [DIM, DIM], BF16, isOutput=False)
    bo_pm = nc.declare_dram_parameter("bo_pm", [128, MT], F32, isOutput=False)
    maskU = nc.declare_dram_parameter("maskU", [128, 128], BF16, isOutput=False)
    ident = nc.declare_dram_parameter("ident", [128, 128], BF16, isOutput=False)
    outT = nc.declare_dram_parameter("outT", [B, DIM, TOW], F32, isOutput=True)

    with tile.TileContext(nc) as tc:
        with (
            tc.tile_pool(name="wpool", bufs=1) as wpool,
            tc.tile_pool(name="xpool", bufs=2) as xpool,
            tc.tile_pool(name="spool", bufs=2) as spool,
            tc.tile_pool(name="opool", bufs=3) as opool,
            tc.tile_pool(name="psum", bufs=2, space="PSUM") as psum,
        ):
            # ---- replicated constants (loaded once) ----
            wq_sb = wpool.tile([128, KT, DIM], BF16)
            wkv_sb = wpool.tile([128, KT, 2 * DIM], BF16)
            wo_sb = wpool.tile([128, KT, DIM], BF16)
            bo_sb = wpool.tile([128, MT], F32)
            mask_sb = wpool.tile([128, 128], BF16)
            id_sb = wpool.tile([128, 128], BF16)
            for k in range(KT):
                nc.sync.dma_start(out=wq_sb[:, k, :], in_=wq[k * 128:(k + 1) * 128, :])
                nc.sync.dma_start(out=wkv_sb[:, k, :], in_=wkv[k * 128:(k + 1) * 128, :])
                nc.sync.dma_start(out=wo_sb[:, k, :], in_=wo[k * 128:(k + 1) * 128, :])
            nc.sync.dma_start(out=bo_sb, in_=bo_pm[:])
            nc.sync.dma_start(out=mask_sb, in_=maskU[:])
            nc.sync.dma_start(out=id_sb, in_=ident[:])

            # ---- persistent activation buffers (reused across batches) ----
            qT = wpool.tile([128, MT, TOW], BF16)            # q, feature-major
            kTt = wpool.tile([128, MT, TH], BF16)            # k, feature-major
            v_sb = wpool.tile([128, NW + 1, H, DH + 1], BF16)  # v + ones col
            exp_sb = wpool.tile([128, 2, 4, 4, 2, W], BF16)  # probs ring
            ao_tok = wpool.tile([128, 2, H, DH], BF16)       # attn out ring
            aoT = wpool.tile([128, KT, TOW], BF16)           # attn out, fmajor
            # softmax-denominator ones column (written once)
            nc.vector.memset(v_sb[:, :, :, DH], 1.0)
            if not ATT_TP:
                nc.vector.memset(aoT, 0.25)

            for b in [bb % B for bb in range(B * REPEAT)]:
                # ---- load xT slice (feature-major, with halo) ----
                x_sb = xpool.tile([128, KT, TH], BF16, tag="x")
                nc.gpsimd.dma_start(
                    out=x_sb[:],
                    in_=xT[b].rearrange("(k p) t -> p k t", p=128),
                )

                # ---- q projection, feature-major ----
                for m in range(MT):
                    for c in range(2):
                        ps = psum.tile([128, 512], F32, tag="proj")
                        for k in range(KT):
                            nc.tensor.matmul(
                                ps,
                                lhsT=wq_sb[:, k, m * 128:(m + 1) * 128],
                                rhs=x_sb[:, k, W + c * 512:W + (c + 1) * 512],
                                start=(k == 0),
                                stop=(k == KT - 1),
                            )
                        nc.vector.tensor_copy(
                            out=qT[:, m, c * 512:(c + 1) * 512], in_=ps
                        )

                # ---- k projection, feature-major (incl. halo) ----
                for m in range(MT):
                    for c in range(3):
                        ps = psum.tile([128, 384], F32, tag="proj")
                        for k in range(KT):
                            nc.tensor.matmul(
                                ps,
                                lhsT=wkv_sb[:, k, m * 128:(m + 1) * 128],
                                rhs=x_sb[:, k, c * 384:(c + 1) * 384],
                                start=(k == 0),
                                stop=(k == KT - 1),
                            )
                        nc.vector.tensor_copy(
                            out=kTt[:, m, c * 384:(c + 1) * 384], in_=ps
                        )

                # ---- v projection, token-major (incl. halo) ----
                for wi in range(NW + 1):
                    for c in range(2):
                        ps = psum.tile([128, 512], F32, tag="proj")
                        for k in range(KT):
                            nc.tensor.matmul(
                                ps,
                                lhsT=x_sb[:, k, wi * 128:(wi + 1) * 128],
                                rhs=wkv_sb[:, k, DIM + c * 512:DIM + (c + 1) * 512],
                                start=(k == 0),
                                stop=(k == KT - 1),
                            )
                        nc.vector.tensor_copy(
                            out=v_sb[:, wi, c * 8:(c + 1) * 8, 0:DH],
                            in_=ps.rearrange("p (h d) -> p h d", d=DH),
                        )

                # ---- attention over k-windows u; q-window w = u-1 ----
                for u in range(NW + 1):
                    if not ATT_SC:
                        break
                    w = u - 1
                    for g in range(4):
                        sc = psum.tile([128, 4, 2, W], F32, tag="sc")
                        # Heads are processed r=0 pair first, then r=64 pair,
                        # with the full-128-row mask matmuls in between: the
                        # PE faults on back-to-back 64-row stationaries at
                        # different base partitions, so every 0<->64 switch
                        # must be separated by a 128-row matmul (the baseline
                        # kernel relied on this ordering implicitly).
                        for rpair in range(2):
                            hhs = (rpair, rpair + 2)
                            for hh in hhs:
                                h = 4 * g + hh
                                m, r = h // 2, (h % 2) * 64
                                kw = kTt[r:r + 64, m, u * W:(u + 1) * W]
                                if u == 0:
                                    nc.tensor.matmul(
                                        sc[:, hh, 1, :], lhsT=kw,
                                        rhs=qT[r:r + 64, m, 0:W],
                                        start=True, stop=True,
                                    )
                                elif u == NW:
                                    nc.tensor.matmul(
                                        sc[:, hh, 0, :], lhsT=kw,
                                        rhs=qT[r:r + 64, m, (u - 1) * W:u * W],
                                        start=True, stop=True,
                                    )
                                else:
                                    nc.tensor.matmul(
                                        sc[:, hh, :, :], lhsT=kw,
                                        rhs=qT[r:r + 64, m,
                                               (u - 1) * W:(u + 1) * W],
                                        start=True, stop=True,
                                    )
                            # causal mask on cur-role block (u>=1); at u=0 it
                            # just fills the unused slot (exp'd to 0, never
                            # consumed).  Always emitted: it is the 128-row
                            # separator the PE needs.
                            for hh in hhs:
                                if u == 0:
                                    nc.tensor.matmul(
                                        sc[:, hh, 0, :], lhsT=mask_sb,
                                        rhs=id_sb, start=True, stop=True,
                                    )
                                else:
                                    nc.tensor.matmul(
                                        sc[:, hh, 0, :], lhsT=mask_sb,
                                        rhs=id_sb, start=False, stop=False,
                                        skip_group_check=True,
                                    )
                        # exp to bf16 SBUF; per head pair (one PSUM bank
                        # each, contiguous [128, 512] APs).  At u=0/NW the
                        # unused half holds stale PSUM junk; its exp lands in
                        # exp_sb slots that are never consumed.
                        eo = exp_sb[:, u % 2, g]
                        for hp in range(2):
                            sl = slice(2 * hp, 2 * hp + 2)
                            nc.scalar.activation(
                                out=eo[:, sl, :, :], in_=sc[:, sl, :, :],
                                func=AF.Exp, bias=0.0, scale=SCALE,
                            )

                        if u >= 1 and ATT_PV:
                            pvt = psum.tile([128, 4, DH + 1], F32, tag="pv",
                                            bufs=1)
                            for hh in range(4):
                                h = 4 * g + hh
                                for hf in (0, 1):
                                    nc.tensor.matmul(
                                        pvt[:, hh, :],
                                        lhsT=exp_sb[:, (w + hf) % 2, g, hh,
                                                    1 - hf, :],
                                        rhs=v_sb[:, w + hf, h, :],
                                        start=(hf == 0),
                                        stop=(hf == 1),
                                    )
                            recip = spool.tile([128, 4], F32, tag="recip")
                            nc.vector.reciprocal(out=recip, in_=pvt[:, :, DH])
                            for hh in range(4):
                                h = 4 * g + hh
                                nc.vector.tensor_scalar_mul(
                                    out=ao_tok[:, w % 2, h, :],
                                    in0=pvt[:, hh, 0:DH],
                                    scalar1=recip[:, hh:hh + 1],
                                )

                    if u >= 1 and ATT_PV and ATT_TP:
                        # transpose attn out of window w to feature-major
                        tp = psum.tile([128, KT, W], BF16, tag="tp", bufs=1)
                        for k2 in range(KT):
                            nc.tensor.transpose(
                                out=tp[:, k2, :],
                                in_=ao_tok[:, w % 2, 2 * k2:2 * k2 + 2, :],
                                identity=id_sb,
                            )
                        nc.vector.tensor_copy(
                            out=aoT[:, :, w * W:(w + 1) * W], in_=tp
                        )

                # ---- output projection + bias ----
                if OUTPROJ:
                    for c in range(2):
                        for m in range(MT):
                            ps = psum.tile([128, 512], F32, tag="proj")
                            for k in range(KT):
                                nc.tensor.matmul(
                                    ps,
                                    lhsT=wo_sb[:, k, m * 128:(m + 1) * 128],
                                    rhs=aoT[:, k, c * 512:(c + 1) * 512],
                                    start=(k == 0),
                                    stop=(k == KT - 1),
                                )
                            osb = opool.tile([128, 512], F32, tag="outsb")
                            nc.vector.tensor_scalar_add(
                                out=osb, in0=ps, scalar1=bo_sb[:, m:m + 1]
                            )
                            nc.sync.dma_start(
                                out=outT[b, m * 128:(m + 1) * 128,
                                         c * 512:(c + 1) * 512],
                                in_=osb,
                            )
                else:
                    for m in range(MT):
                        for c in range(2):
                            osb = opool.tile([128, 512], F32, tag="outsb")
                            nc.vector.tensor_copy(
                                out=osb, in_=qT[:, m, c * 512:(c + 1) * 512]
                            )
                            nc.sync.dma_start(
                                out=outT[b, m * 128:(m + 1) * 128,
                                         c * 512:(c + 1) * 512],
                                in_=osb,
                            )
    nc.compile()
    return nc


_NC_CACHE = None


def _get_nc():
    global _NC_CACHE
    if _NC_CACHE is None:
        _NC_CACHE = _build_bass()
    return _NC_CACHE


def kernel(x, wq, wkv, wo, bo):
    global LAST_RESULT
    bfd = ml_dtypes.bfloat16
    x = np.asarray(x, np.float32)
    wq_b = np.asarray(wq, np.float32).astype(bfd)
    wkv_b = np.asarray(wkv, np.float32).astype(bfd)
    wo_b = np.asarray(wo, np.float32).astype(bfd)
    bo_pm = np.ascontiguousarray(
        np.asarray(bo, np.float32).reshape(MT, 128).T
    )
    # maskU[i, jc] = -1e30 where cur-window col jc > row i (causal); added to
    # the transposed scores via sc[jc, i] += (maskU @ I)[jc, i] = maskU[i, jc]
    maskU = np.where(
        np.arange(W)[None, :] > np.arange(W)[:, None], -1e30, 0.0
    ).astype(bfd)
    ident = np.eye(128, dtype=bfd)

    xb = x.astype(bfd)
    in_maps = []
    for c in range(NCORES):
        lo, hi = c * TOW - W, (c + 1) * TOW
        if c == 0:
            sl = np.concatenate(
                [np.zeros((B, W, DIM), bfd), xb[:, :hi]], axis=1
            )
        else:
            sl = xb[:, lo:hi]
        xT_c = np.ascontiguousarray(sl.transpose(0, 2, 1))  # [B, DIM, TH]
        in_maps.append(
            dict(xT=xT_c, wq=wq_b, wkv=wkv_b, wo=wo_b, bo_pm=bo_pm,
                 maskU=maskU, ident=ident)
        )

    nc = _get_nc()
    res = run_bass_kernel_spmd(
        nc, in_maps, list(range(NCORES)), trace=TRACE, **TRACE_KW
    )
    if TRACE:
        LAST_RESULT = res
    out = np.empty((B, N, DIM), np.float32)
    for c in range(NCORES):
        out[:, c * TOW:(c + 1) * TOW, :] = res.results[c]["outT"].transpose(0, 2, 1)
    return out
